# revision 1
# baseline (speedup 1.0000x reference)
"""HMLSTMOutput fused MLP kernel for Trainium2, 8-core data-parallel.

Network (per token, N = B*T = 32768 tokens):
  g  = sigmoid(x @ Wg.T)                  [N, 3]
  hg = x * repeat(g, 512)                 [N, 1536]   (per-layer gating)
  s  = hg @ Wr.T + be.sum(0); he = relu   [N, 1024]   (Wr = We merged)
  a1 = tanh(he @ W1.T + b1)               [N, 1024]
  a2 = tanh(a1 @ W2.T + b2)               [N, 1024]
  out = a2 @ Wo.T + bo                    [N, 512]

Sharding: tokens split across 8 cores (4096 tokens/core), weights replicated.
On-chip layout: activations feature-major [feat, tok] so every layer's matmul
contracts over the partition dim with pre-transposed weights as the stationary
operand; the final layer uses the activation as the stationary operand to come
back out token-major. All matmuls in bf16 (fp32 PSUM accumulate).
"""

import numpy as np
import ml_dtypes

bf16 = ml_dtypes.bfloat16

# dims (hardcoded for this problem)
B, T = 64, 512
L, IN = 3, 512
D = L * IN            # 1536
E = 1024
H1, H2 = 1024, 1024
O = 512
NCORES = 8
NTOK = B * T // NCORES   # 4096 tokens per core
CHUNK = 512              # tokens per on-chip chunk
NCHUNK = NTOK // CHUNK   # 8
P = 128
KD, KE, KH = D // P, E // P, H2 // P   # 12, 8, 8

_BUILT = {}


def _split_excess_waits(nc, mybir, keep=1):
    """This container's walrus rejects >~1 sync wait on CTRL-class ops (the
    Tile exit drain collects one wait per unobserved proc). Hoist excess
    waits onto single-wait NoOps on the same engine, preserving order."""
    cnt = 0
    for f in nc.m.functions:
        for bb in f.blocks:
            new, changed = [], False
            for inst in bb.instructions:
                si = getattr(inst, "sync_info", None)
                if si is not None and si.on_wait and len(si.on_wait) > keep:
                    waits = list(si.on_wait)
                    excess, waits = waits[:-keep], waits[-keep:]
                    for w in excess:
                        cnt += 1
                        new.append(mybir.InstNoOp(
                            name=f"I-waitsplit-{cnt}", engine=inst.engine,
                            ins=[], outs=[],
                            sync_info=mybir.SyncInfo(on_wait=[w], on_update=[])))
                    inst.sync_info = mybir.SyncInfo(
                        on_wait=waits, on_update=list(si.on_update))
                    changed = True
                new.append(inst)
            if changed:
                bb.instructions = new
    return cnt


def _build():
    import concourse.bass as bass
    import concourse.mybir as mybir
    import concourse.tile as tile

    dt = mybir.dt
    AF = mybir.ActivationFunctionType

    nc = bass.Bass()
    xT_d = nc.dram_tensor("xT", [D, NTOK], dt.bfloat16, kind="ExternalInput")
    wg_d = nc.dram_tensor("wgT", [D, L], dt.bfloat16, kind="ExternalInput")
    wr_d = nc.dram_tensor("wrT", [D, E], dt.bfloat16, kind="ExternalInput")
    w1_d = nc.dram_tensor("w1T", [E, H1], dt.bfloat16, kind="ExternalInput")
    w2_d = nc.dram_tensor("w2T", [H1, H2], dt.bfloat16, kind="ExternalInput")
    wo_d = nc.dram_tensor("woT", [H2, O], dt.bfloat16, kind="ExternalInput")
    bs_d = nc.dram_tensor("bs", [P, KE], dt.float32, kind="ExternalInput")
    b1_d = nc.dram_tensor("b1r", [P, KE], dt.float32, kind="ExternalInput")
    b2_d = nc.dram_tensor("b2r", [P, KE], dt.float32, kind="ExternalInput")
    bor_d = nc.dram_tensor("bor", [P, O], dt.float32, kind="ExternalInput")
    sel_d = nc.dram_tensor("sel", [L, L * P], dt.bfloat16, kind="ExternalInput")
    out_d = nc.dram_tensor("out", [NTOK, O], dt.float32, kind="ExternalOutput")

    with tile.TileContext(nc) as tc:
        with (
            tc.tile_pool(name="wpool", bufs=1) as wp,
            tc.tile_pool(name="xpool", bufs=2) as xp,
            tc.tile_pool(name="hpool", bufs=2) as hp,
            tc.tile_pool(name="apool", bufs=2) as apool,
            tc.tile_pool(name="opool", bufs=6) as op,
            tc.tile_pool(name="gpool", bufs=2) as gp,
            tc.tile_pool(name="pmm", bufs=4, space="PSUM") as pp,
            tc.tile_pool(name="pg", bufs=2, space="PSUM") as pgp,
            tc.tile_pool(name="prep", bufs=2, space="PSUM") as prp,
        ):
            # resident weights / constants
            wg_sb = wp.tile([P, KD, L], dt.bfloat16)
            nc.sync.dma_start(wg_sb[:], wg_d[:].rearrange("(ko p) m -> p ko m", p=P))
            wr_sb = wp.tile([P, KD, E], dt.bfloat16)
            nc.sync.dma_start(wr_sb[:], wr_d[:].rearrange("(ko p) m -> p ko m", p=P))
            w1_sb = wp.tile([P, KE, H1], dt.bfloat16)
            nc.sync.dma_start(w1_sb[:], w1_d[:].rearrange("(ko p) m -> p ko m", p=P))
            w2_sb = wp.tile([P, KE, H2], dt.bfloat16)
            nc.sync.dma_start(w2_sb[:], w2_d[:].rearrange("(ko p) m -> p ko m", p=P))
            wo_sb = wp.tile([P, KH, O], dt.bfloat16)
            nc.sync.dma_start(wo_sb[:], wo_d[:].rearrange("(ko p) m -> p ko m", p=P))
            bs_sb = wp.tile([P, KE], dt.float32)
            nc.sync.dma_start(bs_sb[:], bs_d[:])
            b1_sb = wp.tile([P, KE], dt.float32)
            nc.sync.dma_start(b1_sb[:], b1_d[:])
            b2_sb = wp.tile([P, KE], dt.float32)
            nc.sync.dma_start(b2_sb[:], b2_d[:])
            bor_sb = wp.tile([P, O], dt.float32)
            nc.sync.dma_start(bor_sb[:], bor_d[:])
            sel_sb = wp.tile([L, L * P], dt.bfloat16)
            nc.sync.dma_start(sel_sb[:], sel_d[:])

            xT_r = xT_d[:].rearrange("(ko p) t -> p ko t", p=P)

            for c in range(NCHUNK):
                t0 = c * CHUNK
                # x chunk, feature-major (pre-transposed on host)
                xt = xp.tile([P, KD, CHUNK], dt.bfloat16, tag="xt")
                nc.sync.dma_start(xt[:], xT_r[:, :, t0:t0 + CHUNK])

                # gate logits g = Wg.T-contraction over all 1536 features
                g_ps = pgp.tile([L, CHUNK], dt.float32, tag="g_ps")
                for k in range(KD):
                    nc.tensor.matmul(g_ps[:], wg_sb[:, k, :], xt[:, k, :],
                                     start=(k == 0), stop=(k == KD - 1))
                g_sb = gp.tile([L, CHUNK], dt.bfloat16, tag="g_sb")
                nc.scalar.activation(g_sb[:], g_ps[:], AF.Sigmoid)

                # gated input: broadcast g row l to 128 partitions via PE,
                # then elementwise multiply the 4 k-tiles of that layer block
                hg = hp.tile([P, KD, CHUNK], dt.bfloat16, tag="hg")
                for l in range(L):
                    rep = prp.tile([P, CHUNK], dt.float32, tag="rep")
                    nc.tensor.matmul(rep[:], sel_sb[:, l * P:(l + 1) * P], g_sb[:],
                                     start=True, stop=True)
                    for kk in range(KD // L):
                        k = l * (KD // L) + kk
                        nc.vector.tensor_mul(hg[:, k, :], xt[:, k, :], rep[:])

                # L1: 1536 -> 1024, relu, += be.sum(0)
                a1 = apool.tile([P, KE, CHUNK], dt.bfloat16, tag="a1")
                for m in range(KE):
                    ps = pp.tile([P, CHUNK], dt.float32, tag="mm")
                    for k in range(KD):
                        nc.tensor.matmul(ps[:], wr_sb[:, k, m * P:(m + 1) * P],
                                         hg[:, k, :], start=(k == 0), stop=(k == KD - 1))
                    nc.scalar.activation(a1[:, m, :], ps[:], AF.Relu,
                                         bias=bs_sb[:, m:m + 1])

                # L2: 1024 -> 1024, tanh
                a2 = apool.tile([P, KE, CHUNK], dt.bfloat16, tag="a2")
                for m in range(KE):
                    ps = pp.tile([P, CHUNK], dt.float32, tag="mm")
                    for k in range(KE):
                        nc.tensor.matmul(ps[:], w1_sb[:, k, m * P:(m + 1) * P],
                                         a1[:, k, :], start=(k == 0), stop=(k == KE - 1))
                    nc.scalar.activation(a2[:, m, :], ps[:], AF.Tanh,
                                         bias=b1_sb[:, m:m + 1])

                # L3: 1024 -> 1024, tanh
                a3 = apool.tile([P, KE, CHUNK], dt.bfloat16, tag="a3")
                for m in range(KE):
                    ps = pp.tile([P, CHUNK], dt.float32, tag="mm")
                    for k in range(KE):
                        nc.tensor.matmul(ps[:], w2_sb[:, k, m * P:(m + 1) * P],
                                         a2[:, k, :], start=(k == 0), stop=(k == KE - 1))
                    nc.scalar.activation(a3[:, m, :], ps[:], AF.Tanh,
                                         bias=b2_sb[:, m:m + 1])

                # L4: 1024 -> 512, token-major out via activation-stationary
                for tt in range(CHUNK // P):
                    ps = pp.tile([P, CHUNK], dt.float32, tag="mm")
                    po = ps[:, :O]
                    for k in range(KH):
                        nc.tensor.matmul(po, a3[:, k, tt * P:(tt + 1) * P],
                                         wo_sb[:, k, :], start=(k == 0), stop=(k == KH - 1))
                    osb = op.tile([P, O], dt.float32, tag="osb")
                    nc.vector.tensor_add(osb[:], po, bor_sb[:])
                    row = t0 + tt * P
                    nc.sync.dma_start(out_d[row:row + P, :], osb[:])

    import concourse.mybir as mybir2
    _split_excess_waits(nc, mybir2)
    return nc


def _get_nc():
    if "nc" not in _BUILT:
        _BUILT["nc"] = _build()
    return _BUILT["nc"]


def kernel(x, Wg, We, be, W1, b1, W2, b2, Wo, bo):
    from concourse.bass_utils import run_bass_kernel_spmd

    x = np.asarray(x, dtype=np.float32)
    Wg = np.asarray(Wg, dtype=np.float32)
    We = np.asarray(We, dtype=np.float32)
    be = np.asarray(be, dtype=np.float32)
    W1 = np.asarray(W1, dtype=np.float32)
    b1 = np.asarray(b1, dtype=np.float32)
    W2 = np.asarray(W2, dtype=np.float32)
    b2 = np.asarray(b2, dtype=np.float32)
    Wo = np.asarray(Wo, dtype=np.float32)
    bo = np.asarray(bo, dtype=np.float32)

    # host-side weight prep (shared across cores)
    Wr = We.transpose(1, 0, 2).reshape(E, D)          # [1024, 1536]
    wgT = np.ascontiguousarray(Wg.T).astype(bf16)     # [1536, 3]
    wrT = np.ascontiguousarray(Wr.T).astype(bf16)     # [1536, 1024]
    w1T = np.ascontiguousarray(W1.T).astype(bf16)     # [1024, 1024]
    w2T = np.ascontiguousarray(W2.T).astype(bf16)     # [1024, 1024]
    woT = np.ascontiguousarray(Wo.T).astype(bf16)     # [1024, 512]
    bs = np.ascontiguousarray(be.sum(0).reshape(KE, P).T)   # [128, 8]
    b1r = np.ascontiguousarray(b1.reshape(KE, P).T)
    b2r = np.ascontiguousarray(b2.reshape(KE, P).T)
    bor = np.ascontiguousarray(np.tile(bo, (P, 1)))          # [128, 512]
    sel = np.zeros((L, L * P), dtype=bf16)
    for l in range(L):
        sel[l, l * P:(l + 1) * P] = 1

    shared = {"wgT": wgT, "wrT": wrT, "w1T": w1T, "w2T": w2T, "woT": woT,
              "bs": bs, "b1r": b1r, "b2r": b2r, "bor": bor, "sel": sel}

    x_flat = x.reshape(B * T, D)
    in_maps = []
    for c in range(NCORES):
        xc = x_flat[c * NTOK:(c + 1) * NTOK].T.astype(bf16)  # [1536, 4096] C-order
        in_maps.append({"xT": np.ascontiguousarray(xc), **shared})

    nc = _get_nc()
    res = run_bass_kernel_spmd(nc, in_maps, core_ids=list(range(NCORES)),
                               trace=False)
    out = np.concatenate([res.results[c]["out"] for c in range(NCORES)], axis=0)
    return out.reshape(B, T, O)


# revision 9
# speedup vs baseline: 1.0875x; 1.0875x over previous
"""HMLSTMOutput fused MLP kernel for Trainium2, 8-core data-parallel.

Network (per token, N = B*T = 32768 tokens):
  g  = sigmoid(x @ Wg.T)                  [N, 3]
  hg = x * repeat(g, 512)                 [N, 1536]   (per-layer gating)
  s  = hg @ Wr.T + be.sum(0); he = relu   [N, 1024]   (Wr = We merged)
  a1 = tanh(he @ W1.T + b1)               [N, 1024]
  a2 = tanh(a1 @ W2.T + b2)               [N, 1024]
  out = a2 @ Wo.T + bo                    [N, 512]

Sharding: tokens split across 8 cores (4096 tokens/core), weights replicated.
On-chip layout: activations feature-major [feat, tok] so every layer's matmul
contracts over the partition dim with pre-transposed weights as the stationary
operand; the final layer uses the activation as the stationary operand to come
back out token-major. All matmuls in bf16 (fp32 PSUM accumulate).
"""

import numpy as np
import ml_dtypes

bf16 = ml_dtypes.bfloat16

# dims (hardcoded for this problem)
B, T = 64, 512
L, IN = 3, 512
D = L * IN            # 1536
E = 1024
H1, H2 = 1024, 1024
O = 512
NCORES = 8
NTOK = B * T // NCORES   # 4096 tokens per core
CHUNK = 512              # tokens per on-chip chunk
NCHUNK = NTOK // CHUNK   # 8
P = 128
KD, KE, KH = D // P, E // P, H2 // P   # 12, 8, 8

_BUILT = {}


def _split_excess_waits(nc, mybir, keep=1):
    """This container's walrus rejects >~1 sync wait on CTRL-class ops (the
    Tile exit drain collects one wait per unobserved proc). Hoist excess
    waits onto single-wait NoOps on the same engine, preserving order."""
    cnt = 0
    for f in nc.m.functions:
        for bb in f.blocks:
            new, changed = [], False
            for inst in bb.instructions:
                si = getattr(inst, "sync_info", None)
                if si is not None and si.on_wait and len(si.on_wait) > keep:
                    waits = list(si.on_wait)
                    excess, waits = waits[:-keep], waits[-keep:]
                    for w in excess:
                        cnt += 1
                        new.append(mybir.InstNoOp(
                            name=f"I-waitsplit-{cnt}", engine=inst.engine,
                            ins=[], outs=[],
                            sync_info=mybir.SyncInfo(on_wait=[w], on_update=[])))
                    inst.sync_info = mybir.SyncInfo(
                        on_wait=waits, on_update=list(si.on_update))
                    changed = True
                new.append(inst)
            if changed:
                bb.instructions = new
    return cnt


def _build():
    import concourse.bass as bass
    import concourse.mybir as mybir
    import concourse.tile as tile

    dt = mybir.dt
    AF = mybir.ActivationFunctionType

    nc = bass.Bass()
    xT_d = nc.dram_tensor("xT", [D, NTOK], dt.bfloat16, kind="ExternalInput")
    wg_d = nc.dram_tensor("wgT", [D, L], dt.bfloat16, kind="ExternalInput")
    wr_d = nc.dram_tensor("wrT", [D, E], dt.bfloat16, kind="ExternalInput")
    w1_d = nc.dram_tensor("w1T", [E, H1], dt.bfloat16, kind="ExternalInput")
    w2_d = nc.dram_tensor("w2T", [H1, H2], dt.bfloat16, kind="ExternalInput")
    wo_d = nc.dram_tensor("woT", [H2, O], dt.bfloat16, kind="ExternalInput")
    bs_d = nc.dram_tensor("bs", [P, KE], dt.float32, kind="ExternalInput")
    b1_d = nc.dram_tensor("b1r", [P, KE], dt.float32, kind="ExternalInput")
    b2_d = nc.dram_tensor("b2r", [P, KE], dt.float32, kind="ExternalInput")
    bor_d = nc.dram_tensor("bor", [P, O], dt.float32, kind="ExternalInput")
    out_d = nc.dram_tensor("out", [NTOK, O], dt.float32, kind="ExternalOutput")

    with tile.TileContext(nc) as tc:
        with (
            tc.tile_pool(name="wpool", bufs=1) as wp,
            tc.tile_pool(name="xpool", bufs=2) as xp,
            tc.tile_pool(name="hpool", bufs=2) as hp,
            tc.tile_pool(name="apool", bufs=2) as apool,
            tc.tile_pool(name="opool", bufs=6) as op,
            tc.tile_pool(name="gpool", bufs=2) as gp,
            tc.tile_pool(name="pmm", bufs=6, space="PSUM") as pp,
            tc.tile_pool(name="pg", bufs=1, space="PSUM") as pgp,
            tc.tile_pool(name="dram", bufs=2, space="DRAM") as dp,
        ):
            # small constants first so chunk-0's gate work can start while the
            # big weight matrices stream in
            wg_sb = wp.tile([P, KD, L], dt.bfloat16)
            nc.sync.dma_start(wg_sb[:], wg_d[:].rearrange("(ko p) m -> p ko m", p=P))
            bs_sb = wp.tile([P, KE], dt.float32)
            nc.sync.dma_start(bs_sb[:], bs_d[:])
            b1_sb = wp.tile([P, KE], dt.float32)
            nc.sync.dma_start(b1_sb[:], b1_d[:])
            b2_sb = wp.tile([P, KE], dt.float32)
            nc.sync.dma_start(b2_sb[:], b2_d[:])
            bor_sb = wp.tile([P, O], dt.float32)
            nc.sync.dma_start(bor_sb[:], bor_d[:])

            xT_r = xT_d[:].rearrange("(ko p) t -> p ko t", p=P)

            def load_x(c):
                # split into k-groups so the gate matmuls can start early
                xt = xp.tile([P, KD, CHUNK], dt.bfloat16, tag="xt", name=f"xt{c}")
                for kg in range(0, KD, 3):
                    nc.sync.dma_start(
                        xt[:, kg:kg + 3, :],
                        xT_r[:, kg:kg + 3, c * CHUNK:(c + 1) * CHUNK])
                return xt

            def gate_logits(c, xt):
                # gate logits: contraction over all 1536 features -> [3, CHUNK]
                g_ps = pgp.tile([L, CHUNK], dt.float32, tag="g_ps", name=f"gps{c}")
                for k in range(KD):
                    nc.tensor.matmul(g_ps[:], wg_sb[:, k, :], xt[:, k, :],
                                     start=(k == 0), stop=(k == KD - 1))
                g_sb = gp.tile([L, CHUNK], dt.bfloat16, tag="g_sb", name=f"gsb{c}")
                nc.scalar.activation(g_sb[:], g_ps[:], AF.Sigmoid)
                # bounce through DRAM to broadcast each gate row to all 128
                # partitions on the (idle) DMA engines, keeping PE out of it
                g_dram = dp.tile([L, CHUNK], dt.bfloat16, tag="g_dram",
                                 name=f"gdram{c}")
                nc.sync.dma_start(g_dram[:], g_sb[:])
                rep = gp.tile([P, L, CHUNK], dt.bfloat16, tag="rep", name=f"rep{c}")
                for l in range(L):
                    nc.sync.dma_start(rep[:, l, :],
                                      g_dram[l:l + 1, :].to_broadcast((P, CHUNK)))
                return rep

            def gate_apply(c, xt, rep):
                # gate the 4 k-tiles of each layer block on DVE
                hg = hp.tile([P, KD, CHUNK], dt.bfloat16, tag="hg", name=f"hg{c}")
                for l in range(L):
                    for kk in range(KD // L):
                        k = l * (KD // L) + kk
                        nc.vector.tensor_mul(hg[:, k, :], xt[:, k, :], rep[:, l, :])
                return hg

            # prologue: chunk 0+1 gate pipeline before the big weight loads,
            # so PE has gate matmuls to chew on while wr streams in
            xt_c = load_x(0)
            g_c = gate_logits(0, xt_c)
            xt_n = load_x(1)
            hg_c = gate_apply(0, xt_c, g_c)
            g_n = gate_logits(1, xt_n)

            # wr split per output column so L1(0) m=0 can start after 384KB
            wr_sb = wp.tile([P, KD, E], dt.bfloat16)
            wr_r = wr_d[:].rearrange("(ko p) m -> p ko m", p=P)
            for m in range(KE):
                nc.sync.dma_start(wr_sb[:, :, m * P:(m + 1) * P],
                                  wr_r[:, :, m * P:(m + 1) * P])
            w1_sb = wp.tile([P, KE, H1], dt.bfloat16)
            nc.sync.dma_start(w1_sb[:], w1_d[:].rearrange("(ko p) m -> p ko m", p=P))
            w2_sb = wp.tile([P, KE, H2], dt.bfloat16)
            nc.sync.dma_start(w2_sb[:], w2_d[:].rearrange("(ko p) m -> p ko m", p=P))
            wo_sb = wp.tile([P, KH, O], dt.bfloat16)
            nc.sync.dma_start(wo_sb[:], wo_d[:].rearrange("(ko p) m -> p ko m", p=P))

            for c in range(NCHUNK):
                t0 = c * CHUNK
                hg = hg_c

                # L1: 1536 -> 1024, relu, += be.sum(0)
                a1 = apool.tile([P, KE, CHUNK], dt.bfloat16, tag="a1", name=f"a1_{c}")
                for m in range(KE):
                    ps = pp.tile([P, CHUNK], dt.float32, tag="mm")
                    for k in range(KD):
                        nc.tensor.matmul(ps[:], wr_sb[:, k, m * P:(m + 1) * P],
                                         hg[:, k, :], start=(k == 0), stop=(k == KD - 1))
                    nc.scalar.activation(a1[:, m, :], ps[:], AF.Relu,
                                         bias=bs_sb[:, m:m + 1])

                # prefetch next chunk's x + gate logits (sigmoid overlaps L2)
                if c + 1 < NCHUNK:
                    if c == 0:
                        xt_c, g_c = xt_n, g_n
                    else:
                        xt_c = load_x(c + 1)
                        g_c = gate_logits(c + 1, xt_c)

                # L2: 1024 -> 1024, tanh
                a2 = apool.tile([P, KE, CHUNK], dt.bfloat16, tag="a2", name=f"a2_{c}")
                for m in range(KE):
                    ps = pp.tile([P, CHUNK], dt.float32, tag="mm")
                    for k in range(KE):
                        nc.tensor.matmul(ps[:], w1_sb[:, k, m * P:(m + 1) * P],
                                         a1[:, k, :], start=(k == 0), stop=(k == KE - 1))
                    nc.scalar.activation(a2[:, m, :], ps[:], AF.Tanh,
                                         bias=b1_sb[:, m:m + 1])

                # next chunk's gating multiplies (DVE work overlaps L3)
                if c + 1 < NCHUNK:
                    hg_c = gate_apply(c + 1, xt_c, g_c)

                # L3: 1024 -> 1024, tanh
                a3 = apool.tile([P, KE, CHUNK], dt.bfloat16, tag="a3", name=f"a3_{c}")
                for m in range(KE):
                    ps = pp.tile([P, CHUNK], dt.float32, tag="mm")
                    for k in range(KE):
                        nc.tensor.matmul(ps[:], w2_sb[:, k, m * P:(m + 1) * P],
                                         a2[:, k, :], start=(k == 0), stop=(k == KE - 1))
                    nc.scalar.activation(a3[:, m, :], ps[:], AF.Tanh,
                                         bias=b2_sb[:, m:m + 1])

                # L4: 1024 -> 512, token-major out via activation-stationary
                for tt in range(CHUNK // P):
                    ps = pp.tile([P, CHUNK], dt.float32, tag="mm")
                    po = ps[:, :O]
                    for k in range(KH):
                        nc.tensor.matmul(po, a3[:, k, tt * P:(tt + 1) * P],
                                         wo_sb[:, k, :], start=(k == 0), stop=(k == KH - 1))
                    osb = op.tile([P, O], dt.float32, tag="osb")
                    nc.vector.tensor_add(osb[:], po, bor_sb[:])
                    row = t0 + tt * P
                    nc.sync.dma_start(out_d[row:row + P, :], osb[:])

    import concourse.mybir as mybir2
    _split_excess_waits(nc, mybir2)
    return nc


def _get_nc():
    if "nc" not in _BUILT:
        _BUILT["nc"] = _build()
    return _BUILT["nc"]


def kernel(x, Wg, We, be, W1, b1, W2, b2, Wo, bo):
    from concourse.bass_utils import run_bass_kernel_spmd

    x = np.asarray(x, dtype=np.float32)
    Wg = np.asarray(Wg, dtype=np.float32)
    We = np.asarray(We, dtype=np.float32)
    be = np.asarray(be, dtype=np.float32)
    W1 = np.asarray(W1, dtype=np.float32)
    b1 = np.asarray(b1, dtype=np.float32)
    W2 = np.asarray(W2, dtype=np.float32)
    b2 = np.asarray(b2, dtype=np.float32)
    Wo = np.asarray(Wo, dtype=np.float32)
    bo = np.asarray(bo, dtype=np.float32)

    # host-side weight prep (shared across cores)
    Wr = We.transpose(1, 0, 2).reshape(E, D)          # [1024, 1536]
    wgT = np.ascontiguousarray(Wg.T).astype(bf16)     # [1536, 3]
    wrT = np.ascontiguousarray(Wr.T).astype(bf16)     # [1536, 1024]
    w1T = np.ascontiguousarray(W1.T).astype(bf16)     # [1024, 1024]
    w2T = np.ascontiguousarray(W2.T).astype(bf16)     # [1024, 1024]
    woT = np.ascontiguousarray(Wo.T).astype(bf16)     # [1024, 512]
    bs = np.ascontiguousarray(be.sum(0).reshape(KE, P).T)   # [128, 8]
    b1r = np.ascontiguousarray(b1.reshape(KE, P).T)
    b2r = np.ascontiguousarray(b2.reshape(KE, P).T)
    bor = np.ascontiguousarray(np.tile(bo, (P, 1)))          # [128, 512]
    shared = {"wgT": wgT, "wrT": wrT, "w1T": w1T, "w2T": w2T, "woT": woT,
              "bs": bs, "b1r": b1r, "b2r": b2r, "bor": bor}

    x_flat = x.reshape(B * T, D)
    in_maps = []
    for c in range(NCORES):
        xc = x_flat[c * NTOK:(c + 1) * NTOK].T.astype(bf16)  # [1536, 4096] C-order
        in_maps.append({"xT": np.ascontiguousarray(xc), **shared})

    nc = _get_nc()
    res = run_bass_kernel_spmd(nc, in_maps, core_ids=list(range(NCORES)),
                               trace=False)
    out = np.concatenate([res.results[c]["out"] for c in range(NCORES)], axis=0)
    return out.reshape(B, T, O)
